# revision 1
# baseline (speedup 1.0000x reference)
"""GNN message passing (gather + weighted segment-sum) on 8 Trainium2 cores.

out[n, :] = sum_{e : dst[e] == n} weight[e] * queue[src[e], :]

Strategy
--------
Edges are sharded by destination window (128 destination nodes per window,
49 windows per core).  Each core:
  * gathers queue[src] rows straight from HBM with `dma_gather`
    (indices are int16, so the 50000-row queue is addressed as two
    parity-interleaved 25000-row strided views: even rows / odd rows),
  * builds a weighted one-hot matrix H[e, j] = weight[e] * (dstoff[e] == j)
    per 128-edge block with a single dual-op tensor_scalar on the DVE,
  * accumulates H.T @ G into a [128, 64] PSUM tile per window on the
    TensorEngine (fp32),
  * copies each finished window to SBUF and DMAs it to its slice of the
    output.

All data-dependent structure (edges per window, padded uniformly across
cores so one SPMD NEFF serves all 8 cores) is computed on the host from the
actual inputs at call time.
"""

import contextlib
import sys

sys.path.insert(0, "/opt/trn_rl_repo")

import ml_dtypes
import numpy as np

import concourse.bass as bass  # noqa: F401
import concourse.mybir as mybir
import concourse.tile as tile
from concourse import bacc
from concourse.bass_utils import run_bass_kernel_spmd

P = 128
N_CORES = 8

N_NODES = 50000
N_EDGES = 800000
D_FEAT = 64


def _plan(n_nodes, n_cores):
    """Windows-per-core and chunking. All cores run the identical program."""
    n_windows = -(-n_nodes // P)
    wpc = -(-n_windows // n_cores)
    # chunk width: largest divisor of wpc that keeps gather tiles a sane size
    cw = max(d for d in range(1, min(wpc, 8) + 1) if wpc % d == 0)
    nchunk = wpc // cw
    return wpc, cw, nchunk


def _host_prep(weight, src, dst, n_nodes, wpc, cw, nchunk, n_cores):
    """Bucket edges by (core, window, src parity); pad uniformly.

    Returns (epw, idx_hbm, aux_hbm):
      idx_hbm [n_cores, nchunk, 2, 128, cw*epw//16] int16  (dma_gather layout)
      aux_hbm [n_cores, nchunk, 128, 4*cw*nb] f32  (dstoff then weight, packed
              so block k of half h of window j sits at column (h*cw+j)*nb+k)
    """
    e = src.shape[0]
    src = np.asarray(src).astype(np.int64).reshape(-1)
    dst = np.asarray(dst).astype(np.int64).reshape(-1)
    wgt = np.asarray(weight, dtype=np.float32).reshape(-1)

    w = dst >> 7
    core = w // wpc
    lw = w - core * wpc
    half = src & 1
    hidx = (src >> 1).astype(np.int16)
    dstoff = (dst & 127).astype(np.float32)

    nbuckets = n_cores * wpc * 2
    key = (core * wpc + lw) * 2 + half
    # secondary sort key: src, for HBM locality within each gather
    order = np.lexsort((src, key))
    counts = np.bincount(key, minlength=nbuckets)
    epw = int(-(-max(int(counts.max()), 1) // P) * P)
    offs = np.zeros(nbuckets + 1, np.int64)
    np.cumsum(counts, out=offs[1:])
    skey = key[order]
    rank = np.arange(e, dtype=np.int64) - offs[skey]
    dest = skey * epw + rank

    # weight split: wgt == w_hi + w_lo with both terms bf16-exact
    w_hi = wgt.astype(ml_dtypes.bfloat16)
    w_lo = (wgt - w_hi.astype(np.float32)).astype(ml_dtypes.bfloat16)

    bf = ml_dtypes.bfloat16
    # pads are trailing -1 indices: the gather ucode trims them (no packets)
    idx_arr = np.full(nbuckets * epw, -1, np.int16)
    dst_arr = np.zeros(nbuckets * epw, bf)
    whi_arr = np.zeros(nbuckets * epw, bf)
    wlo_arr = np.zeros(nbuckets * epw, bf)
    idx_arr[dest] = hidx[order]
    dst_arr[dest] = dstoff[order].astype(bf)  # 0..127, exact in bf16
    whi_arr[dest] = w_hi[order]
    wlo_arr[dest] = w_lo[order]

    nb = epw // P
    big = cw * epw  # indices per chunk-half
    shp = (n_cores, nchunk, cw, 2, epw)
    idx_arr = idx_arr.reshape(shp)
    dst_arr = dst_arr.reshape(shp)
    whi_arr = whi_arr.reshape(shp)
    wlo_arr = wlo_arr.reshape(shp)

    # idx: window-major edge list per (core, chunk, half), wrapped mod 16 and
    # replicated to 128 partitions (8 Q7 cores each read a 16-partition copy).
    a = idx_arr.transpose(0, 1, 3, 2, 4).reshape(n_cores, nchunk, 2, big // 16, 16)
    a = a.transpose(0, 1, 2, 4, 3)  # [.., 16, big//16]
    idx_hbm = np.broadcast_to(
        a[:, :, :, None, :, :], (n_cores, nchunk, 2, 8, 16, big // 16)
    ).reshape(n_cores, nchunk, 2, P, big // 16)
    idx_hbm = np.ascontiguousarray(idx_hbm)

    def pack(x):
        # window-major block columns: col = (j*2 + h)*nb + k
        y = x.reshape(n_cores, nchunk, cw, 2, nb, P)
        y = y.transpose(0, 1, 5, 2, 3, 4)  # [core, chunk, P, j, h, k]
        return y.reshape(n_cores, nchunk, P, 2 * cw * nb)

    aux_hbm = np.concatenate(
        [pack(dst_arr), pack(whi_arr), pack(wlo_arr)], axis=3
    )
    aux_hbm = np.ascontiguousarray(aux_hbm)
    # per-gather valid-edge counts, ordered (chunk, window, half)
    cnt_hbm = np.ascontiguousarray(
        counts.reshape(n_cores, nchunk, cw, 2)
        .reshape(n_cores, 1, nchunk * cw * 2)
        .astype(np.int32)
    )
    return epw, idx_hbm, aux_hbm, cnt_hbm


ALL_PARTS = frozenset({"gather", "dve", "mm", "out"})

TERMS = 3  # hi*hi + hi*lo + lo*hi (lo*lo ~ 4e-6 relative, dropped)


def _build(n_nodes, d, epw, wpc, cw, nchunk, iters=1, parts=ALL_PARTS):
    f32 = mybir.dt.float32
    bf16 = mybir.dt.bfloat16
    nb = epw // P
    big = cw * epw
    bpc = cw * nb  # blocks per half per chunk
    ne = n_nodes // 2
    assert n_nodes % 2 == 0

    nc = bacc.Bacc(
        "TRN2", target_bir_lowering=False, debug=False, num_swdge_queues=4
    )

    # qhl[p] = 256 bf16: [hi(node 2p) | lo(node 2p) | hi(node 2p+1) | lo(node 2p+1)]
    qhl_t = nc.dram_tensor("qhl", [ne, 4 * d], bf16, kind="ExternalInput")
    idx_t = nc.dram_tensor(
        "idx", [nchunk, 2, P, big // 16], mybir.dt.int16, kind="ExternalInput"
    )
    aux_t = nc.dram_tensor("aux", [nchunk, P, 6 * bpc], bf16, kind="ExternalInput")
    iota_t = nc.dram_tensor("iota", [P, P], bf16, kind="ExternalInput")
    cnt_t = nc.dram_tensor(
        "cnt", [1, nchunk * cw * 2], mybir.dt.int32, kind="ExternalInput"
    )
    out_t = nc.dram_tensor("out", [wpc * P, d], f32, kind="ExternalOutput")

    q2 = qhl_t.ap()  # [ne, 4d]
    qviews = [q2[:, 0 : 2 * d], q2[:, 2 * d : 4 * d]]

    with tile.TileContext(nc) as tc:
        gbufs = 6
        with (
            tc.tile_pool(name="const", bufs=1) as cpool,
            tc.tile_pool(name="io", bufs=2) as iopool,
            tc.tile_pool(name="gat", bufs=gbufs) as gpool,
            tc.tile_pool(name="hot", bufs=3) as hpool,
            tc.tile_pool(name="ost", bufs=4) as opool,
            tc.tile_pool(name="ps", bufs=4, space="PSUM") as ppool,
        ):
            iota_f = cpool.tile([P, P], bf16)
            nc.sync.dma_start(out=iota_f[:], in_=iota_t.ap()[:, :])
            cnt = cpool.tile([1, nchunk * cw * 2], mybir.dt.int32)
            nc.sync.dma_start(out=cnt[:], in_=cnt_t.ap()[:, :])
            # pre-zero the gather slots: trimmed (padded) tail positions are
            # never written by the gather, and must not contain NaN patterns
            for h in (0, 1):
                for _ in range(gbufs):
                    gz = gpool.tile([P, nb, 2 * d], bf16, tag=f"g{h}")
                    nc.vector.memset(gz[:], 0)

            loop = tc.For_i(0, iters, 1) if iters > 1 else contextlib.nullcontext()
            with loop:
                for c in range(nchunk):
                    idxs = []
                    for h in (0, 1):
                        it = iopool.tile(
                            [P, big // 16], mybir.dt.int16, tag=f"idx{h}"
                        )
                        nc.sync.dma_start(out=it[:], in_=idx_t.ap()[c, h])
                        idxs.append(it)
                    aux = iopool.tile([P, 6 * bpc], bf16, tag="aux")
                    nc.sync.dma_start(out=aux[:], in_=aux_t.ap()[c])

                    for j in range(cw):
                        gt = []
                        for h in (0, 1):
                            g = gpool.tile([P, nb, 2 * d], bf16, tag=f"g{h}")
                            if "gather" in parts:
                                sl = epw // 16
                                gidx = (c * cw + j) * 2 + h
                                r = nc.alloc_register(mybir.EngineType.Pool)
                                nc.gpsimd.reg_load(
                                    r, cnt[0:1, gidx : gidx + 1]
                                )
                                nc.gpsimd.dma_gather(
                                    out_ap=g[:],
                                    in_ap=qviews[h],
                                    idxs_ap=idxs[h][:, j * sl : (j + 1) * sl],
                                    num_idxs=epw,
                                    num_idxs_reg=r,
                                    elem_size=2 * d,
                                    elem_step=4 * d,
                                    single_packet=False,
                                    queue_num=(2 * j + h) % 4,
                                )
                            elif "seqload" in parts:
                                flat = qhl_t.ap()[0 : P * 64, :].rearrange(
                                    "(p c) d -> p (c d)", p=P
                                )
                                nc.sync.dma_start(
                                    out=g[:].rearrange("p a b -> p (a b)"),
                                    in_=flat[:, 0 : nb * 2 * d],
                                )
                            gt.append(g)

                        ps = ppool.tile([P, d], f32)
                        nbw = 2 * nb  # blocks in this window (both halves)
                        wcol = j * nbw  # first block column of this window

                        def bcast(ap2d, n_mid, mid_is_data):
                            # [P, X] -> [P, n_mid, P] AP; data dim keeps its
                            # step, the other dim gets step 0
                            pairs = list(ap2d.ap)
                            assert len(pairs) == 2
                            if mid_is_data:
                                newp = [pairs[0], [pairs[1][0], n_mid], [0, P]]
                            else:
                                newp = [pairs[0], [0, n_mid], pairs[1]]
                            return bass.AP(ap2d.tensor, ap2d.offset, newp)

                        if "dve" in parts:
                            h01 = hpool.tile([P, nbw, P], bf16, tag="h01")
                            nc.vector.tensor_tensor(
                                out=h01[:],
                                in0=bcast(iota_f[:], nbw, False),
                                in1=bcast(aux[:, wcol : wcol + nbw], nbw, True),
                                op=mybir.AluOpType.is_equal,
                            )
                            hhi = hpool.tile([P, nbw, P], bf16, tag="hhi")
                            nc.vector.tensor_tensor(
                                out=hhi[:],
                                in0=h01[:],
                                in1=bcast(
                                    aux[:, 2 * bpc + wcol : 2 * bpc + wcol + nbw],
                                    nbw,
                                    True,
                                ),
                                op=mybir.AluOpType.mult,
                            )
                            hlo = hpool.tile([P, nbw, P], bf16, tag="hlo")
                            nc.vector.tensor_tensor(
                                out=hlo[:],
                                in0=h01[:],
                                in1=bcast(
                                    aux[:, 4 * bpc + wcol : 4 * bpc + wcol + nbw],
                                    nbw,
                                    True,
                                ),
                                op=mybir.AluOpType.mult,
                            )
                        if "mm" in parts:
                            first = True
                            for h in (0, 1):
                                for k in range(nb):
                                    bi = h * nb + k  # block within window
                                    if "dve" in parts:
                                        hi_ap = hhi[:, bi, :]
                                        lo_ap = hlo[:, bi, :]
                                    else:
                                        hi_ap = lo_ap = iota_f[:]
                                    g_hi = gt[h][:, k, 0:d]
                                    g_lo = gt[h][:, k, d : 2 * d]
                                    terms = [
                                        (hi_ap, g_hi),
                                        (hi_ap, g_lo),
                                        (lo_ap, g_hi),
                                    ]
                                    if TERMS == 4:
                                        terms.append((lo_ap, g_lo))
                                    last_blk = h == 1 and k == nb - 1
                                    for t, (lhs, rhs) in enumerate(terms):
                                        nc.tensor.matmul(
                                            ps[:],
                                            lhsT=lhs,
                                            rhs=rhs,
                                            start=first,
                                            stop=last_blk
                                            and t == len(terms) - 1,
                                        )
                                        first = False
                        wg = c * cw + j
                        if "out" in parts and "mm" in parts:
                            ot = opool.tile([P, d], f32, tag="ot")
                            nc.scalar.copy(ot[:], ps[:])
                            nc.sync.dma_start(
                                out=out_t.ap()[wg * P : (wg + 1) * P, :], in_=ot[:]
                            )
                        elif "dve" in parts and "mm" not in parts:
                            # variant build: keep the one-hots alive (anti-DCE)
                            nc.vector.tensor_tensor(
                                out=hhi[:, 0, :],
                                in0=hhi[:, 0, :],
                                in1=hlo[:, 0, :],
                                op=mybir.AluOpType.max,
                            )
                            nc.sync.dma_start(
                                out=out_t.ap()[wg * P : (wg + 1) * P, :],
                                in_=hhi[:, 0, :].bitcast(f32),
                            )
                        elif "dve" not in parts and "mm" not in parts:
                            # variant build: keep the loads alive (anti-DCE)
                            nc.sync.dma_start(
                                out=out_t.ap()[wg * P : (wg + 1) * P, :],
                                in_=gt[0][:, 0, :].bitcast(f32),
                            )
                            nc.sync.dma_start(
                                out=out_t.ap()[wg * P : (wg + 1) * P, :],
                                in_=gt[1][:, 0, :].bitcast(f32),
                            )
    nc.compile()
    return nc


def _make_inputs(queue, idx_hbm, aux_hbm, cnt_hbm, n_cores):
    bf = ml_dtypes.bfloat16
    q = np.asarray(queue, dtype=np.float32)
    hi = q.astype(bf)
    lo = (q - hi.astype(np.float32)).astype(bf)
    ne, d = q.shape[0] // 2, q.shape[1]
    qhl = np.empty((ne, 4 * d), bf)
    qhl[:, 0:d] = hi[0::2]
    qhl[:, d : 2 * d] = lo[0::2]
    qhl[:, 2 * d : 3 * d] = hi[1::2]
    qhl[:, 3 * d : 4 * d] = lo[1::2]
    iota_np = np.ascontiguousarray(
        np.broadcast_to(np.arange(P, dtype=np.float32), (P, P)).astype(bf)
    )
    return [
        {
            "qhl": qhl,
            "idx": idx_hbm[c],
            "aux": aux_hbm[c],
            "iota": iota_np,
            "cnt": cnt_hbm[c],
        }
        for c in range(n_cores)
    ]


def _run(queue, weight, src, dst, n_nodes, d, n_cores, trace=False, iters=1):
    queue = np.ascontiguousarray(np.asarray(queue, dtype=np.float32))
    wpc, cw, nchunk = _plan(n_nodes, n_cores)
    epw, idx_hbm, aux_hbm, cnt_hbm = _host_prep(
        weight, src, dst, n_nodes, wpc, cw, nchunk, n_cores
    )
    nc = _build(n_nodes, d, epw, wpc, cw, nchunk, iters=iters)
    in_maps = _make_inputs(queue, idx_hbm, aux_hbm, cnt_hbm, n_cores)
    res = run_bass_kernel_spmd(nc, in_maps, core_ids=list(range(n_cores)), trace=trace)
    full = np.concatenate([res.results[c]["out"] for c in range(n_cores)], axis=0)
    return full[:n_nodes], res


def kernel(queue, weight, src, dst):
    out, _ = _run(queue, weight, src, dst, N_NODES, D_FEAT, N_CORES)
    return out



# revision 4
# speedup vs baseline: 1.9896x; 1.9896x over previous
"""GNN message passing (gather + weighted segment-sum) on 8 Trainium2 cores.

out[n, :] = sum_{e : dst[e] == n} weight[e] * queue[src[e], :]

Strategy
--------
Edges are sharded by destination window (128 destination nodes per window,
49 windows per core).  Each core:
  * gathers queue[src] rows straight from HBM with `dma_gather`
    (indices are int16, so the 50000-row queue is addressed as two
    parity-interleaved 25000-row strided views: even rows / odd rows),
  * builds a weighted one-hot matrix H[e, j] = weight[e] * (dstoff[e] == j)
    per 128-edge block with a single dual-op tensor_scalar on the DVE
    (is_equal with the per-edge dst offset, then mult by the per-edge
    weight; both are per-partition scalar columns),
  * accumulates H.T @ G into a [128, 64] PSUM tile per window on the
    TensorEngine (bf16 inputs, fp32 accumulate),
  * copies each finished window to SBUF and DMAs it to its slice of the
    output.

All data-dependent structure (edges per window, padded uniformly across
cores so one SPMD NEFF serves all 8 cores) is computed on the host from the
actual inputs at call time.
"""

import contextlib
import sys

sys.path.insert(0, "/opt/trn_rl_repo")

import ml_dtypes
import numpy as np

import concourse.bass as bass  # noqa: F401
import concourse.mybir as mybir
import concourse.tile as tile
from concourse import bacc
from concourse.bass_utils import run_bass_kernel_spmd

P = 128
N_CORES = 8

N_NODES = 50000
N_EDGES = 800000
D_FEAT = 64

SINGLE_PACKET = False
DMA_SCRATCH = 16384


def _plan(n_nodes, n_cores):
    """Windows-per-core and chunking. All cores run the identical program."""
    n_windows = -(-n_nodes // P)
    wpc = -(-n_windows // n_cores)
    # chunk width: largest divisor of wpc that keeps gather tiles a sane size
    cw = max(d for d in range(1, min(wpc, 8) + 1) if wpc % d == 0)
    nchunk = wpc // cw
    return wpc, cw, nchunk


def _host_prep(weight, src, dst, n_nodes, wpc, cw, nchunk, n_cores):
    """Bucket edges by (core, window, src parity); pad uniformly.

    Returns (epw, idx_hbm, dst_hbm, wh_hbm, cnt_hbm):
      idx_hbm [n_cores, nchunk, 2, 128, cw*epw//16] int16  (dma_gather layout)
      dst_hbm [n_cores, nchunk, 128, 2*cw*nb] f32   (dst offsets, packed so
              block k of half h of window j sits at column (j*2+h)*nb+k)
      wh_hbm  [n_cores, nchunk, 128, 2*cw*nb] bf16  (weights, same packing)
    """
    e = src.shape[0]
    src = np.asarray(src).astype(np.int64).reshape(-1)
    dst = np.asarray(dst).astype(np.int64).reshape(-1)
    wgt = np.asarray(weight, dtype=np.float32).reshape(-1)

    w = dst >> 7
    core = w // wpc
    lw = w - core * wpc
    half = src & 1
    hidx = (src >> 1).astype(np.int16)
    dstoff = (dst & 127).astype(np.float32)

    nbuckets = n_cores * wpc * 2
    key = (core * wpc + lw) * 2 + half
    # secondary sort key: src, for HBM locality within each gather
    order = np.lexsort((src, key))
    counts = np.bincount(key, minlength=nbuckets)
    epw = int(-(-max(int(counts.max()), 1) // P) * P)
    offs = np.zeros(nbuckets + 1, np.int64)
    np.cumsum(counts, out=offs[1:])
    skey = key[order]
    rank = np.arange(e, dtype=np.int64) - offs[skey]
    dest = skey * epw + rank

    bf = ml_dtypes.bfloat16
    # pads are trailing -1 indices: the gather ucode trims them (no packets)
    idx_arr = np.full(nbuckets * epw, -1, np.int16)
    dst_arr = np.zeros(nbuckets * epw, np.float32)
    wh_arr = np.zeros(nbuckets * epw, np.float32)
    idx_arr[dest] = hidx[order]
    dst_arr[dest] = dstoff[order]
    wh_arr[dest] = wgt[order]

    nb = epw // P
    big = cw * epw  # indices per chunk-half
    shp = (n_cores, nchunk, cw, 2, epw)
    idx_arr = idx_arr.reshape(shp)
    dst_arr = dst_arr.reshape(shp)
    wh_arr = wh_arr.reshape(shp)

    # idx: window-major edge list per (core, chunk, half), wrapped mod 16 and
    # replicated to 128 partitions (8 Q7 cores each read a 16-partition copy).
    a = idx_arr.transpose(0, 1, 3, 2, 4).reshape(n_cores, nchunk, 2, big // 16, 16)
    a = a.transpose(0, 1, 2, 4, 3)  # [.., 16, big//16]
    idx_hbm = np.broadcast_to(
        a[:, :, :, None, :, :], (n_cores, nchunk, 2, 8, 16, big // 16)
    ).reshape(n_cores, nchunk, 2, P, big // 16)
    idx_hbm = np.ascontiguousarray(idx_hbm)

    def pack(x):
        # window-major block columns: col = (j*2 + h)*nb + k
        y = x.reshape(n_cores, nchunk, cw, 2, nb, P)
        y = y.transpose(0, 1, 5, 2, 3, 4)  # [core, chunk, P, j, h, k]
        return np.ascontiguousarray(y.reshape(n_cores, nchunk, P, 2 * cw * nb))

    dst_hbm = pack(dst_arr)
    wh_hbm = pack(wh_arr)
    # per-gather valid-edge counts, ordered (chunk, window, half)
    cnt_hbm = np.ascontiguousarray(
        counts.reshape(n_cores, nchunk, cw, 2)
        .reshape(n_cores, 1, nchunk * cw * 2)
        .astype(np.int32)
    )
    return epw, idx_hbm, dst_hbm, wh_hbm, cnt_hbm


ALL_PARTS = frozenset({"gather", "dve", "mm", "out"})


def _build(n_nodes, d, epw, wpc, cw, nchunk, iters=1, parts=ALL_PARTS):
    f32 = mybir.dt.float32
    bf16 = mybir.dt.bfloat16
    nb = epw // P
    big = cw * epw
    bpc = 2 * cw * nb  # block columns per chunk
    ne = n_nodes // 2
    assert n_nodes % 2 == 0

    nc = bacc.Bacc(
        "TRN2",
        target_bir_lowering=False,
        debug=False,
        num_swdge_queues=4,
        dynamic_dma_scratch_size=DMA_SCRATCH,
    )

    # qhl[p] = 256 bf16: [hi(node 2p) | lo(node 2p) | hi(node 2p+1) | lo(node 2p+1)]
    qhl_t = nc.dram_tensor("qhl", [ne, 4 * d], bf16, kind="ExternalInput")
    idx_t = nc.dram_tensor(
        "idx", [nchunk, 2, P, big // 16], mybir.dt.int16, kind="ExternalInput"
    )
    dstf_t = nc.dram_tensor("dstf", [nchunk, P, bpc], f32, kind="ExternalInput")
    wh_t = nc.dram_tensor("wh", [nchunk, P, bpc], f32, kind="ExternalInput")
    iota_t = nc.dram_tensor("iota", [P, P], bf16, kind="ExternalInput")
    cnt_t = nc.dram_tensor(
        "cnt", [1, nchunk * cw * 2], mybir.dt.int32, kind="ExternalInput"
    )
    out_t = nc.dram_tensor("out", [wpc * P, d], f32, kind="ExternalOutput")

    q2 = qhl_t.ap()  # [ne, 4d]
    qviews = [q2[:, 0 : 2 * d], q2[:, 2 * d : 4 * d]]

    with tile.TileContext(nc) as tc:
        gbufs = 6
        with (
            tc.tile_pool(name="const", bufs=1) as cpool,
            tc.tile_pool(name="io", bufs=2) as iopool,
            tc.tile_pool(name="gat", bufs=gbufs) as gpool,
            tc.tile_pool(name="hot", bufs=3) as hpool,
            tc.tile_pool(name="ost", bufs=4) as opool,
            tc.tile_pool(name="ps", bufs=4, space="PSUM") as ppool,
        ):
            iota_f = cpool.tile([P, P], bf16)
            nc.sync.dma_start(out=iota_f[:], in_=iota_t.ap()[:, :])
            cnt = cpool.tile([1, nchunk * cw * 2], mybir.dt.int32)
            nc.sync.dma_start(out=cnt[:], in_=cnt_t.ap()[:, :])
            # pre-zero the gather slots: trimmed (padded) tail positions are
            # never written by the gather, and must not contain NaN patterns
            for h in (0, 1):
                for _ in range(gbufs):
                    gz = gpool.tile([P, nb, 2 * d], bf16, tag=f"g{h}")
                    nc.vector.memset(gz[:], 0)

            loop = tc.For_i(0, iters, 1) if iters > 1 else contextlib.nullcontext()
            with loop:
                for c in range(nchunk):
                    idxs = []
                    for h in (0, 1):
                        it = iopool.tile(
                            [P, big // 16], mybir.dt.int16, tag=f"idx{h}"
                        )
                        nc.sync.dma_start(out=it[:], in_=idx_t.ap()[c, h])
                        idxs.append(it)
                    dstf = iopool.tile([P, bpc], f32, tag="dstf")
                    nc.sync.dma_start(out=dstf[:], in_=dstf_t.ap()[c])
                    wh = iopool.tile([P, bpc], f32, tag="wh")
                    nc.sync.dma_start(out=wh[:], in_=wh_t.ap()[c])

                    for j in range(cw):
                        gt = []
                        for h in (0, 1):
                            g = gpool.tile([P, nb, 2 * d], bf16, tag=f"g{h}")
                            if "gather" in parts:
                                sl = epw // 16
                                gidx = (c * cw + j) * 2 + h
                                r = nc.alloc_register(mybir.EngineType.Pool)
                                nc.gpsimd.reg_load(
                                    r, cnt[0:1, gidx : gidx + 1]
                                )
                                nc.gpsimd.dma_gather(
                                    out_ap=g[:],
                                    in_ap=qviews[h],
                                    idxs_ap=idxs[h][:, j * sl : (j + 1) * sl],
                                    num_idxs=epw,
                                    num_idxs_reg=r,
                                    elem_size=2 * d,
                                    elem_step=4 * d,
                                    single_packet=SINGLE_PACKET,
                                    queue_num=(2 * j + h) % 4,
                                )
                            elif "seqload" in parts:
                                flat = qhl_t.ap()[0 : P * 64, :].rearrange(
                                    "(p c) d -> p (c d)", p=P
                                )
                                nc.sync.dma_start(
                                    out=g[:].rearrange("p a b -> p (a b)"),
                                    in_=flat[:, 0 : nb * 2 * d],
                                )
                            gt.append(g)

                        ps = ppool.tile([P, d], f32)
                        nbw = 2 * nb  # blocks in this window (both halves)
                        wcol = j * nbw  # first block column of this window

                        if "dve" in parts:
                            hw_ = hpool.tile([P, nbw, P], bf16, tag="hw")
                            for bi in range(nbw):
                                col = wcol + bi
                                nc.vector.tensor_scalar(
                                    out=hw_[:, bi, :],
                                    in0=iota_f[:],
                                    scalar1=dstf[:, col : col + 1],
                                    scalar2=wh[:, col : col + 1],
                                    op0=mybir.AluOpType.is_equal,
                                    op1=mybir.AluOpType.mult,
                                )
                        if "mm" in parts:
                            for bi in range(nbw):
                                h = bi // nb
                                k = bi - h * nb
                                lhs = (
                                    hw_[:, bi, :]
                                    if "dve" in parts
                                    else iota_f[:]
                                )
                                nc.tensor.matmul(
                                    ps[:],
                                    lhsT=lhs,
                                    rhs=gt[h][:, k, 0:d],
                                    start=bi == 0,
                                    stop=bi == nbw - 1,
                                )
                        wg = c * cw + j
                        if "out" in parts and "mm" in parts:
                            ot = opool.tile([P, d], f32, tag="ot")
                            nc.scalar.copy(ot[:], ps[:])
                            nc.sync.dma_start(
                                out=out_t.ap()[wg * P : (wg + 1) * P, :], in_=ot[:]
                            )
                        elif "dve" in parts and "mm" not in parts:
                            # variant build: keep the one-hots alive (anti-DCE)
                            nc.sync.dma_start(
                                out=out_t.ap()[wg * P : (wg + 1) * P, :],
                                in_=hw_[:, 0, :].bitcast(f32),
                            )
                        elif "dve" not in parts and "mm" not in parts:
                            # variant build: keep the loads alive (anti-DCE)
                            nc.sync.dma_start(
                                out=out_t.ap()[wg * P : (wg + 1) * P, :],
                                in_=gt[0][:, 0, :].bitcast(f32),
                            )
                            nc.sync.dma_start(
                                out=out_t.ap()[wg * P : (wg + 1) * P, :],
                                in_=gt[1][:, 0, :].bitcast(f32),
                            )
    nc.compile()
    return nc


def _make_inputs(queue, idx_hbm, dst_hbm, wh_hbm, cnt_hbm, n_cores):
    bf = ml_dtypes.bfloat16
    q = np.asarray(queue, dtype=np.float32)
    hi = q.astype(bf)
    lo = (q - hi.astype(np.float32)).astype(bf)
    ne, d = q.shape[0] // 2, q.shape[1]
    qhl = np.empty((ne, 4 * d), bf)
    qhl[:, 0:d] = hi[0::2]
    qhl[:, d : 2 * d] = lo[0::2]
    qhl[:, 2 * d : 3 * d] = hi[1::2]
    qhl[:, 3 * d : 4 * d] = lo[1::2]
    iota_np = np.ascontiguousarray(
        np.broadcast_to(np.arange(P, dtype=np.float32), (P, P)).astype(bf)
    )
    return [
        {
            "qhl": qhl,
            "idx": idx_hbm[c],
            "dstf": dst_hbm[c],
            "wh": wh_hbm[c],
            "iota": iota_np,
            "cnt": cnt_hbm[c],
        }
        for c in range(n_cores)
    ]


def _run(queue, weight, src, dst, n_nodes, d, n_cores, trace=False, iters=1):
    queue = np.ascontiguousarray(np.asarray(queue, dtype=np.float32))
    wpc, cw, nchunk = _plan(n_nodes, n_cores)
    epw, idx_hbm, dst_hbm, wh_hbm, cnt_hbm = _host_prep(
        weight, src, dst, n_nodes, wpc, cw, nchunk, n_cores
    )
    nc = _build(n_nodes, d, epw, wpc, cw, nchunk, iters=iters)
    in_maps = _make_inputs(queue, idx_hbm, dst_hbm, wh_hbm, cnt_hbm, n_cores)
    res = run_bass_kernel_spmd(nc, in_maps, core_ids=list(range(n_cores)), trace=trace)
    full = np.concatenate([res.results[c]["out"] for c in range(n_cores)], axis=0)
    return full[:n_nodes], res


def kernel(queue, weight, src, dst):
    out, _ = _run(queue, weight, src, dst, N_NODES, D_FEAT, N_CORES)
    return out


# revision 14
# speedup vs baseline: 3.7969x; 1.9084x over previous
"""GNN message passing (gather + weighted segment-sum) on 8 Trainium2 cores.

out[n, :] = sum_{e : dst[e] == n} weight[e] * queue[src[e], :]

Strategy
--------
Edges are sharded by destination window (128 destination nodes per window,
49 windows per core).  Each core:
  * gathers queue[src] rows straight from HBM with `dma_gather`
    (indices are int16, so the 50000-row queue is addressed as two
    parity-interleaved 25000-row strided views: even rows / odd rows),
  * builds a weighted one-hot matrix H[e, j] = weight[e] * (dstoff[e] == j)
    per 128-edge block with a single dual-op tensor_scalar on the DVE
    (is_equal with the per-edge dst offset, then mult by the per-edge
    weight; both are per-partition scalar columns),
  * accumulates H.T @ G into a [128, 64] PSUM tile per window on the
    TensorEngine (bf16 inputs, fp32 accumulate),
  * copies each finished window to SBUF and DMAs it to its slice of the
    output.

All data-dependent structure (edges per window, padded uniformly across
cores so one SPMD NEFF serves all 8 cores) is computed on the host from the
actual inputs at call time.
"""

import contextlib
import sys

sys.path.insert(0, "/opt/trn_rl_repo")

import ml_dtypes
import numpy as np

import concourse.bass as bass  # noqa: F401
import concourse.mybir as mybir
import concourse.tile as tile
from concourse import bacc
from concourse.bass_utils import run_bass_kernel_spmd

P = 128
N_CORES = 8

N_NODES = 50000
N_EDGES = 800000
D_FEAT = 64

SINGLE_PACKET = False
DMA_SCRATCH = 16384


def _plan(n_nodes, n_cores):
    """Windows-per-core and chunking. All cores run the identical program."""
    n_windows = -(-n_nodes // P)
    wpc = -(-n_windows // n_cores)
    # chunk width: largest divisor of wpc that keeps gather tiles a sane size
    cw = max(d for d in range(1, min(wpc, 8) + 1) if wpc % d == 0)
    nchunk = wpc // cw
    return wpc, cw, nchunk


def _host_prep(weight, src, dst, n_nodes, wpc, cw, nchunk, n_cores):
    """Bucket edges by (core, window, src parity); pad uniformly.

    Returns (epw, idx_hbm, dst_hbm, wh_hbm, cnt_hbm):
      idx_hbm [n_cores, nchunk, 2, 128, cw*epw//16] int16  (dma_gather layout)
      dst_hbm [n_cores, nchunk, 128, 2*cw*nb] f32   (dst offsets, packed so
              block k of half h of window j sits at column (j*2+h)*nb+k)
      wh_hbm  [n_cores, nchunk, 128, 2*cw*nb] bf16  (weights, same packing)
    """
    e = src.shape[0]
    src = np.asarray(src).astype(np.int64).reshape(-1)
    dst = np.asarray(dst).astype(np.int64).reshape(-1)
    wgt = np.asarray(weight, dtype=np.float32).reshape(-1)

    w = dst >> 7
    core = w // wpc
    lw = w - core * wpc
    half = src & 1
    hidx = (src >> 1).astype(np.int16)
    dstoff = (dst & 127).astype(np.float32)

    nbuckets = n_cores * wpc * 2
    key = (core * wpc + lw) * 2 + half
    # secondary sort key: src, for HBM locality within each gather
    order = np.lexsort((src, key))
    counts = np.bincount(key, minlength=nbuckets)
    epw = int(-(-max(int(counts.max()), 1) // P) * P)
    offs = np.zeros(nbuckets + 1, np.int64)
    np.cumsum(counts, out=offs[1:])
    skey = key[order]
    rank = np.arange(e, dtype=np.int64) - offs[skey]
    dest = skey * epw + rank

    bf = ml_dtypes.bfloat16
    # pads are trailing -1 indices: the gather ucode trims them (no packets)
    idx_arr = np.full(nbuckets * epw, -1, np.int16)
    dst_arr = np.zeros(nbuckets * epw, bf)
    wh_arr = np.zeros(nbuckets * epw, bf)
    idx_arr[dest] = hidx[order]
    dst_arr[dest] = dstoff[order].astype(bf)  # 0..127, exact in bf16
    wh_arr[dest] = wgt[order].astype(bf)

    nb = epw // P
    big = cw * epw  # indices per chunk-half
    shp = (n_cores, nchunk, cw, 2, epw)
    idx_arr = idx_arr.reshape(shp)
    dst_arr = dst_arr.reshape(shp)
    wh_arr = wh_arr.reshape(shp)

    # idx: window-major edge list per (core, chunk, half), wrapped mod 16 and
    # replicated to 128 partitions (8 Q7 cores each read a 16-partition copy).
    a = idx_arr.transpose(0, 1, 3, 2, 4).reshape(n_cores, nchunk, 2, big // 16, 16)
    a = a.transpose(0, 1, 2, 4, 3)  # [.., 16, big//16]
    idx_hbm = np.broadcast_to(
        a[:, :, :, None, :, :], (n_cores, nchunk, 2, 8, 16, big // 16)
    ).reshape(n_cores, nchunk, 2, P, big // 16)
    idx_hbm = np.ascontiguousarray(idx_hbm)

    def pack(x):
        # window-major block columns: col = (j*2 + h)*nb + k
        y = x.reshape(n_cores, nchunk, cw, 2, nb, P)
        y = y.transpose(0, 1, 5, 2, 3, 4)  # [core, chunk, P, j, h, k]
        return np.ascontiguousarray(y.reshape(n_cores, nchunk, P, 2 * cw * nb))

    dst_hbm = pack(dst_arr)
    wh_hbm = pack(wh_arr)
    # per-gather valid-edge counts, ordered (chunk, window, half)
    cnt_hbm = np.ascontiguousarray(
        counts.reshape(n_cores, nchunk, cw, 2)
        .reshape(n_cores, 1, nchunk * cw * 2)
        .astype(np.int32)
    )
    return epw, idx_hbm, dst_hbm, wh_hbm, cnt_hbm


ALL_PARTS = frozenset({"gather", "dve", "mm", "out"})


def _build(n_nodes, d, epw, wpc, cw, nchunk, iters=1, parts=ALL_PARTS):
    f32 = mybir.dt.float32
    bf16 = mybir.dt.bfloat16
    nb = epw // P
    big = cw * epw
    bpc = 2 * cw * nb  # block columns per chunk
    ne = n_nodes // 2
    assert n_nodes % 2 == 0

    nc = bacc.Bacc(
        "TRN2",
        target_bir_lowering=False,
        debug=False,
        num_swdge_queues=4,
        dynamic_dma_scratch_size=DMA_SCRATCH,
    )

    # qhl[p] = 256 bf16: [hi(node 2p) | lo(node 2p) | hi(node 2p+1) | lo(node 2p+1)]
    qhl_t = nc.dram_tensor("qhl", [ne, 4 * d], bf16, kind="ExternalInput")
    idx_t = nc.dram_tensor(
        "idx", [nchunk, 2, P, big // 16], mybir.dt.int16, kind="ExternalInput"
    )
    dstf_t = nc.dram_tensor("dstf", [nchunk, P, bpc], bf16, kind="ExternalInput")
    wh_t = nc.dram_tensor("wh", [nchunk, P, bpc], bf16, kind="ExternalInput")
    nbw = 2 * nb
    iota_t = nc.dram_tensor("iota", [P, P * nbw], bf16, kind="ExternalInput")
    cnt_t = nc.dram_tensor(
        "cnt", [1, nchunk * cw * 2], mybir.dt.int32, kind="ExternalInput"
    )
    out_t = nc.dram_tensor("out", [wpc * P, d], f32, kind="ExternalOutput")

    q2 = qhl_t.ap()  # [ne, 4d]
    qviews = [q2[:, 0 : 2 * d], q2[:, 2 * d : 4 * d]]

    with tile.TileContext(nc) as tc:
        gbufs = 6
        with (
            tc.tile_pool(name="const", bufs=1) as cpool,
            tc.tile_pool(name="io", bufs=2) as iopool,
            tc.tile_pool(name="gat", bufs=gbufs) as gpool,
            tc.tile_pool(name="hot", bufs=3) as hpool,
            tc.tile_pool(name="ost", bufs=4) as opool,
            tc.tile_pool(name="ps", bufs=4, space="PSUM") as ppool,
        ):
            # iota3[p, q, b] = q for every block column b (materialized so the
            # one-hot tensor_tensor ops have unit-stride last dims -> 2x DVE)
            iota3 = cpool.tile([P, P, nbw], bf16)
            nc.sync.dma_start(
                out=iota3[:].rearrange("p a b -> p (a b)"), in_=iota_t.ap()[:, :]
            )
            iota_f = cpool.tile([P, P], bf16)  # variant builds only
            nc.sync.dma_start(out=iota_f[:], in_=iota_t.ap()[:, 0:P])
            cnt = cpool.tile([1, nchunk * cw * 2], mybir.dt.int32)
            nc.sync.dma_start(out=cnt[:], in_=cnt_t.ap()[:, :])
            # pre-zero the gather slots: trimmed (padded) tail positions are
            # never written by the gather, and must not contain NaN patterns
            for h in (0, 1):
                for _ in range(gbufs):
                    gz = gpool.tile([P, nb, 2 * d], bf16, tag=f"g{h}")
                    nc.vector.memset(gz[:], 0)

            loop = tc.For_i(0, iters, 1) if iters > 1 else contextlib.nullcontext()
            with loop:
                for c in range(nchunk):
                    idxs = []
                    for h in (0, 1):
                        it = iopool.tile(
                            [P, big // 16], mybir.dt.int16, tag=f"idx{h}"
                        )
                        nc.sync.dma_start(out=it[:], in_=idx_t.ap()[c, h])
                        idxs.append(it)
                    dstf = iopool.tile([P, bpc], bf16, tag="dstf")
                    nc.sync.dma_start(out=dstf[:], in_=dstf_t.ap()[c])
                    wh = iopool.tile([P, bpc], bf16, tag="wh")
                    nc.sync.dma_start(out=wh[:], in_=wh_t.ap()[c])

                    for j in range(cw):
                        gt = []
                        for h in (0, 1):
                            g = gpool.tile([P, nb, 2 * d], bf16, tag=f"g{h}")
                            if "gather" in parts:
                                sl = epw // 16
                                gidx = (c * cw + j) * 2 + h
                                r = nc.alloc_register(mybir.EngineType.Pool)
                                nc.gpsimd.reg_load(
                                    r, cnt[0:1, gidx : gidx + 1]
                                )
                                nc.gpsimd.dma_gather(
                                    out_ap=g[:],
                                    in_ap=qviews[h],
                                    idxs_ap=idxs[h][:, j * sl : (j + 1) * sl],
                                    num_idxs=epw,
                                    num_idxs_reg=r,
                                    elem_size=2 * d,
                                    elem_step=4 * d,
                                    single_packet=SINGLE_PACKET,
                                    queue_num=(2 * j + h) % 4,
                                )
                            elif "seqload" in parts:
                                flat = qhl_t.ap()[0 : P * 64, :].rearrange(
                                    "(p c) d -> p (c d)", p=P
                                )
                                nc.sync.dma_start(
                                    out=g[:].rearrange("p a b -> p (a b)"),
                                    in_=flat[:, 0 : nb * 2 * d],
                                )
                            gt.append(g)

                        ps = ppool.tile([P, d], f32)
                        wcol = j * nbw  # first block column of this window

                        def bcast_mid(ap2d):
                            # [P, nbw] -> [P, P, nbw] with the middle (q) dim
                            # broadcast (step 0); last dim keeps unit stride
                            pairs = list(ap2d.ap)
                            assert len(pairs) == 2
                            newp = [pairs[0], [0, P], pairs[1]]
                            return bass.AP(ap2d.tensor, ap2d.offset, newp)

                        if "dve" in parts:
                            # H[e, q, b] = (q == dstoff[e,b]) * w[e,b], bf16.
                            # All operands have unit-stride last dims -> 2x DVE
                            h01 = hpool.tile([P, P, nbw], bf16, tag="h01")
                            nc.vector.tensor_tensor(
                                out=h01[:],
                                in0=iota3[:],
                                in1=bcast_mid(dstf[:, wcol : wcol + nbw]),
                                op=mybir.AluOpType.is_equal,
                            )
                            hw_ = hpool.tile([P, P, nbw], bf16, tag="hw")
                            nc.vector.tensor_tensor(
                                out=hw_[:],
                                in0=h01[:],
                                in1=bcast_mid(wh[:, wcol : wcol + nbw]),
                                op=mybir.AluOpType.mult,
                            )
                        if "mm" in parts:
                            for bi in range(nbw):
                                h = bi // nb
                                k = bi - h * nb
                                lhs = (
                                    hw_[:, :, bi]
                                    if "dve" in parts
                                    else iota_f[:]
                                )
                                nc.tensor.matmul(
                                    ps[:],
                                    lhsT=lhs,
                                    rhs=gt[h][:, k, 0:d],
                                    start=bi == 0,
                                    stop=bi == nbw - 1,
                                )
                        wg = c * cw + j
                        if "out" in parts and "mm" in parts:
                            ot = opool.tile([P, d], f32, tag="ot")
                            nc.scalar.copy(ot[:], ps[:])
                            nc.sync.dma_start(
                                out=out_t.ap()[wg * P : (wg + 1) * P, :], in_=ot[:]
                            )
                        elif "dve" in parts and "mm" not in parts:
                            # variant build: keep the one-hots alive (anti-DCE)
                            nc.sync.dma_start(
                                out=out_t.ap()[wg * P : (wg + 1) * P, :],
                                in_=hw_[:].rearrange("p a b -> p (a b)")[
                                    :, 0:128
                                ].bitcast(f32),
                            )
                        elif "dve" not in parts and "mm" not in parts:
                            # variant build: keep the loads alive (anti-DCE)
                            nc.sync.dma_start(
                                out=out_t.ap()[wg * P : (wg + 1) * P, :],
                                in_=gt[0][:, 0, :].bitcast(f32),
                            )
                            nc.sync.dma_start(
                                out=out_t.ap()[wg * P : (wg + 1) * P, :],
                                in_=gt[1][:, 0, :].bitcast(f32),
                            )
    nc.compile()
    return nc


def _make_inputs(queue, idx_hbm, dst_hbm, wh_hbm, cnt_hbm, n_cores, epw):
    bf = ml_dtypes.bfloat16
    q = np.asarray(queue, dtype=np.float32)
    hi = q.astype(bf)
    lo = (q - hi.astype(np.float32)).astype(bf)
    ne, d = q.shape[0] // 2, q.shape[1]
    qhl = np.empty((ne, 4 * d), bf)
    qhl[:, 0:d] = hi[0::2]
    qhl[:, d : 2 * d] = lo[0::2]
    qhl[:, 2 * d : 3 * d] = hi[1::2]
    qhl[:, 3 * d : 4 * d] = lo[1::2]
    nbw = 2 * (epw // P)
    # iota3[p, q*nbw + b] = q  (the materialized per-window iota cube)
    iota_np = np.ascontiguousarray(
        np.broadcast_to(
            np.repeat(np.arange(P, dtype=np.float32), nbw)[None, :], (P, P * nbw)
        ).astype(bf)
    )
    return [
        {
            "qhl": qhl,
            "idx": idx_hbm[c],
            "dstf": dst_hbm[c],
            "wh": wh_hbm[c],
            "iota": iota_np,
            "cnt": cnt_hbm[c],
        }
        for c in range(n_cores)
    ]


def _run(queue, weight, src, dst, n_nodes, d, n_cores, trace=False, iters=1):
    queue = np.ascontiguousarray(np.asarray(queue, dtype=np.float32))
    wpc, cw, nchunk = _plan(n_nodes, n_cores)
    epw, idx_hbm, dst_hbm, wh_hbm, cnt_hbm = _host_prep(
        weight, src, dst, n_nodes, wpc, cw, nchunk, n_cores
    )
    nc = _build(n_nodes, d, epw, wpc, cw, nchunk, iters=iters)
    in_maps = _make_inputs(queue, idx_hbm, dst_hbm, wh_hbm, cnt_hbm, n_cores, epw)
    res = run_bass_kernel_spmd(nc, in_maps, core_ids=list(range(n_cores)), trace=trace)
    full = np.concatenate([res.results[c]["out"] for c in range(n_cores)], axis=0)
    return full[:n_nodes], res


def kernel(queue, weight, src, dst):
    out, _ = _run(queue, weight, src, dst, N_NODES, D_FEAT, N_CORES)
    return out
